# revision 1
# baseline (speedup 1.0000x reference)
"""TRN2 Bass kernel for the vq_codebook problem (nn_DNN_34497177321482).

kernel(**inputs) -> np.ndarray  [full-shape in, full-shape out]

Strategy (8 NeuronCores, data-parallel over batch; 64 batches/core):
  - Host packs the active (mask>=1) history positions per core into tiles of
    128 rows; embedding_table (pre-scaled x1024) is gathered on-device via
    indirect DMA.
  - Distances: sadj[r,n] = ||c_n||^2/2 - x_r.c_n (same argmin as full
    squared distance).  The GEMM runs on the PE at bf16 speed with full
    fp32-level accuracy via an fp16 hi/lo 3-term split
    (xh.ch + xh.cl + xl.ch); row-min via DVE reduce; the one-hot is
    eqm1 = Sign(min - sadj) in {-1,0} on the Scalar engine.
  - Per-batch reductions as PE matmuls against a 0/1 membership matrix S:
    cnt' = S^T eqm1 (counts minus batch-size) and hist = S^T x.
    Masked rows quantize to the min-norm code n0: host adds (L-cnt_b) at
    column n0; the constant cnt_b*colsum(cb)@W1 term is folded into the bias.
  - stage2: cntT @ codebook (fp16 pair) -> vq_sum; then
    [vq_mean, hist_mean] @ W_enc + bias on-device; outputs gathered on host.
"""

import sys

sys.path.insert(0, "/opt/trn_rl_repo")

import numpy as np

import concourse.bacc as bacc
import concourse.bass as bass
import concourse.tile as tile
import concourse.mybir as mybir
from concourse.bass_utils import run_bass_kernel_spmd
from concourse.masks import make_identity

F32 = mybir.dt.float32
F16 = mybir.dt.float16
I32 = mybir.dt.int32

V, D, K, L, B = 100000, 256, 2048, 200, 512
N_CORES = 8
BL = B // N_CORES
KC = D // 128          # main-GEMM contraction chunks
NC = K // 512          # distance n-chunks
SC = K // 128          # stage2 contraction chunks
XC = (2 * D) // 128    # final dense contraction chunks
SCALE = 1024.0

_program_cache = {}


def _build_program(T):
    nc = bacc.Bacc("TRN2", target_bir_lowering=False, debug=False,
                   enable_asserts=False, num_devices=N_CORES)

    def din(name, shape, dt):
        return nc.dram_tensor(name, shape, dt, kind="ExternalInput").ap()

    emb_d = din("emb", [V, D], F32)
    ids_d = din("ids", [128, T], I32)
    s16_d = din("s16", [128, T * BL], F16)
    normsb_d = din("normsb", [128, K], F32)
    chT_d = din("chT", [128, KC * K], F16)
    clT_d = din("clT", [128, KC * K], F16)
    ch2_d = din("ch2", [128, SC * D], F16)
    cl2_d = din("cl2", [128, SC * D], F16)
    rvq_d = din("rvq", [BL, 1], F32)
    rhist_d = din("rhist", [BL, 1], F32)
    corr_d = din("corr", [BL, 1], F32)
    g01_d = din("g01", [128, 2 * D], F32)
    wT_d = din("wT", [128, XC * D], F32)
    bias_d = din("bias", [BL, D], F32)
    out_d = nc.dram_tensor("out", [BL, D], F32, kind="ExternalOutput").ap()

    def emit(tc, n0):
        const = tc.alloc_tile_pool(name="const", bufs=1)
        ppersist = tc.alloc_tile_pool(name="ppersist", bufs=1, space="PSUM")

        ident = const.tile([128, 128], F32, name="ident")
        make_identity(nc, ident[:])

        ids_sb = const.tile([128, T], I32, name="ids_sb")
        nc.gpsimd.dma_start(ids_sb[:], ids_d)
        p_gx = tc.alloc_tile_pool(name="gx", bufs=4)
        pregath = {}
        for t in range(min(2, T)):
            gx = p_gx.tile([128, D], F32, tag="gx", name=f"gx{t}")
            nc.sync.dma_start(gx[:], g01_d[:, t * D:(t + 1) * D])
            pregath[t] = gx
        chT_sb = const.tile([128, KC * K], F16, name="chT_sb")
        for kc in range(KC):
            nc.sync.dma_start(chT_sb[:, kc * K:(kc + 1) * K],
                              chT_d[:, kc * K:(kc + 1) * K])
        clT_sb = const.tile([128, KC * K], F16, name="clT_sb")
        for kc in range(KC):
            nc.sync.dma_start(clT_sb[:, kc * K:(kc + 1) * K],
                              clT_d[:, kc * K:(kc + 1) * K])
        normsb_sb = const.tile([128, K], F32, name="normsb_sb")
        nc.sync.dma_start(normsb_sb[:], normsb_d)
        s16_sb = const.tile([128, T * BL], F16, name="s16_sb")
        nc.sync.dma_start(s16_sb[:], s16_d)
        # late consts: tiles now, DMAs after the tile loop
        ch2_sb = const.tile([128, SC * D], F16, name="ch2_sb")
        cl2_sb = const.tile([128, SC * D], F16, name="cl2_sb")
        rvq_sb = const.tile([BL, 1], F32, name="rvq_sb")
        rhist_sb = const.tile([BL, 1], F32, name="rhist_sb")
        corr_sb = const.tile([BL, 1], F32, name="corr_sb")
        wT_sb = const.tile([128, XC * D], F32, name="wT_sb")
        bias_sb = const.tile([BL, D], F32, name="bias_sb")

        cnt_ps = ppersist.tile([128, K // 2], F32, space="PSUM", name="cnt_ps")
        hist_ps = ppersist.tile([BL, D], F32, space="PSUM", name="hist_ps")

        p_xtps = tc.alloc_tile_pool(name="xtps", bufs=2, space="PSUM")
        p_xt = tc.alloc_tile_pool(name="xt", bufs=2)
        p_gh = tc.alloc_tile_pool(name="gh", bufs=2)
        p_sps = tc.alloc_tile_pool(name="sps", bufs=3, space="PSUM")
        p_sadj = tc.alloc_tile_pool(name="sadj", bufs=3)
        p_eq = tc.alloc_tile_pool(name="eq", bufs=2)
        p_m = tc.alloc_tile_pool(name="m", bufs=2)

        for t in range(T):
            if t in pregath:
                gx = pregath[t]
            else:
                gx = p_gx.tile([128, D], F32, tag="gx", name=f"gx{t}")
                nc.gpsimd.indirect_dma_start(
                    out=gx[:], out_offset=None, in_=emb_d,
                    in_offset=bass.IndirectOffsetOnAxis(ap=ids_sb[:, t:t + 1],
                                                        axis=0),
                )
            xt_ps = p_xtps.tile([128, D], F32, tag="xtps", space="PSUM",
                                name=f"xtps{t}")
            for kc in range(KC):
                nc.tensor.transpose(xt_ps[:, kc * 128:(kc + 1) * 128],
                                    gx[:, kc * 128:(kc + 1) * 128], ident[:])
            xh = p_xt.tile([128, D], F16, tag="xh", name=f"xh{t}")
            nc.scalar.copy(xh[:], xt_ps[:])
            xl = p_xt.tile([128, D], F16, tag="xl", name=f"xl{t}")
            nc.vector.tensor_tensor(out=xl[:], in0=xt_ps[:], in1=xh[:],
                                    op=mybir.AluOpType.subtract)
            gh = p_gh.tile([128, D], F16, tag="gh", name=f"gh{t}")
            nc.gpsimd.tensor_copy(gh[:], gx[:])
            gl = p_gh.tile([128, D], F16, tag="gl", name=f"gl{t}")
            nc.gpsimd.tensor_tensor(out=gl[:], in0=gx[:], in1=gh[:],
                                    op=mybir.AluOpType.subtract)

            sadj = p_sadj.tile([128, K], F32, tag="sadj", name=f"sadj{t}")
            m_parts = p_m.tile([128, max(NC, 8)], F32, tag="mparts",
                               name=f"mp{t}")
            for nch in range(NC):
                sl = slice(nch * 512, (nch + 1) * 512)
                s_ps = p_sps.tile([128, 512], F32, tag="sps", space="PSUM",
                                  name=f"sps{t}_{nch}")
                i = 0
                for kc in range(KC):
                    ksl = slice(kc * 128, (kc + 1) * 128)
                    csl = slice(kc * K + nch * 512, kc * K + (nch + 1) * 512)
                    for lhs, rhs in ((xh, chT_sb), (xh, clT_sb), (xl, chT_sb)):
                        nc.tensor.matmul(s_ps[:], lhs[:, ksl], rhs[:, csl],
                                         start=(i == 0), stop=(i == 3 * KC - 1))
                        i += 1
                nc.vector.tensor_tensor(out=sadj[:, sl], in0=normsb_sb[:, sl],
                                        in1=s_ps[:], op=mybir.AluOpType.subtract)
                nc.vector.tensor_reduce(out=m_parts[:, nch:nch + 1],
                                        in_=sadj[:, sl],
                                        axis=mybir.AxisListType.X,
                                        op=mybir.AluOpType.min)
            m_min = p_m.tile([128, 1], F32, tag="m", name=f"m{t}")
            nc.vector.tensor_reduce(out=m_min[:], in_=m_parts[:, :NC],
                                    axis=mybir.AxisListType.X,
                                    op=mybir.AluOpType.min)

            eq = p_eq.tile([128, K], F16, tag="eq", name=f"eq{t}")
            nc.scalar.activation(eq[:], sadj[:],
                                 mybir.ActivationFunctionType.Sign,
                                 bias=m_min[:], scale=-1.0)

            ssl = slice(t * BL, (t + 1) * BL)
            for nch in range(NC):
                sl = slice(nch * 512, (nch + 1) * 512)
                po = 0 if nch < NC // 2 else 64
                psl = slice((nch % (NC // 2)) * 512, (nch % (NC // 2)) * 512 + 512)
                nc.tensor.matmul(cnt_ps[po:po + BL, psl], s16_sb[:, ssl],
                                 eq[:, sl], start=(t == 0), stop=(t == T - 1))
            nc.tensor.matmul(hist_ps[:], s16_sb[:, ssl], gh[:],
                             start=(t == 0), stop=False)
            nc.tensor.matmul(hist_ps[:], s16_sb[:, ssl], gl[:],
                             start=False, stop=(t == T - 1))

        nc.sync.dma_start(ch2_sb[:], ch2_d)
        nc.sync.dma_start(cl2_sb[:], cl2_d)
        nc.sync.dma_start(rvq_sb[:], rvq_d)
        nc.sync.dma_start(rhist_sb[:], rhist_d)
        nc.sync.dma_start(corr_sb[:], corr_d)
        nc.sync.dma_start(wT_sb[:], wT_d)
        nc.sync.dma_start(bias_sb[:], bias_d)

        for p in (p_m, p_eq, p_sadj, p_sps, p_gh, p_xt, p_xtps, p_gx):
            p.release()

        # ---- final phase ----
        fin = tc.alloc_tile_pool(name="fin", bufs=1)
        pfinA = tc.alloc_tile_pool(name="pfinA", bufs=1, space="PSUM")

        cntT = fin.tile([BL, K], F32, name="cntT")
        for nch in range(NC):
            sl = slice(nch * 512, (nch + 1) * 512)
            po = 0 if nch < NC // 2 else 64
            psl = slice((nch % (NC // 2)) * 512, (nch % (NC // 2)) * 512 + 512)
            nc.vector.tensor_copy(cntT[:, sl], cnt_ps[po:po + BL, psl])
            if nch * 512 <= n0 < (nch + 1) * 512:
                nc.vector.tensor_tensor(
                    out=cntT[:, n0:n0 + 1], in0=cntT[:, n0:n0 + 1],
                    in1=corr_sb[:], op=mybir.AluOpType.add)

        vq_ps = pfinA.tile([BL, D], F32, tag="vqps", space="PSUM", name="vqps")
        for kc in range(SC):
            ctp = pfinA.tile([128, BL], F32, tag="ctp", space="PSUM", bufs=2,
                             name=f"ctp{kc}")
            nc.tensor.transpose(ctp[:], cntT[:, kc * 128:(kc + 1) * 128],
                                ident[:BL, :BL])
            cc = fin.tile([128, BL], F16, tag=f"cc{kc % 2}", name=f"cc{kc}")
            nc.vector.tensor_copy(cc[:], ctp[:])
            dsl = slice(kc * D, (kc + 1) * D)
            nc.tensor.matmul(vq_ps[:], cc[:], ch2_sb[:, dsl],
                             start=(kc == 0), stop=False)
            nc.tensor.matmul(vq_ps[:], cc[:], cl2_sb[:, dsl],
                             start=False, stop=(kc == SC - 1))

        x_sb = fin.tile([BL, 2 * D], F32, name="x_sb")
        nc.vector.tensor_scalar_mul(x_sb[:, 0:D], vq_ps[:], rvq_sb[:])
        nc.vector.tensor_scalar_mul(x_sb[:, D:2 * D], hist_ps[:], rhist_sb[:])

        pfinA.release()
        pfinB = tc.alloc_tile_pool(name="pfinB", bufs=1, space="PSUM")
        out_ps = pfinB.tile([BL, D], F32, tag="outps", space="PSUM",
                            name="outps")
        for c in range(XC):
            xtp = pfinB.tile([128, BL], F32, tag="xtp", space="PSUM", bufs=2,
                             name=f"xtp{c}")
            nc.tensor.transpose(xtp[:], x_sb[:, c * 128:(c + 1) * 128],
                                ident[:BL, :BL])
            xc = fin.tile([128, BL], F32, tag=f"xc{c % 2}", name=f"xc{c}")
            nc.vector.tensor_copy(xc[:], xtp[:])
            nc.tensor.matmul(out_ps[:], xc[:], wT_sb[:, c * D:(c + 1) * D],
                             start=(c == 0), stop=(c == XC - 1))

        out_sb = fin.tile([BL, D], F32, name="out_sb")
        nc.vector.tensor_tensor(out=out_sb[:], in0=out_ps[:], in1=bias_sb[:],
                                op=mybir.AluOpType.add)
        nc.sync.dma_start(out_d, out_sb[:])

        for p in (pfinB, fin, ppersist, const):
            p.release()

    return nc, emit


def _get_program(T, n0):
    key = (T, n0)
    if key not in _program_cache:
        nc, emit = _build_program(T)
        with tile.TileContext(nc) as tc:
            emit(tc, n0)
        nc.compile()
        _program_cache[key] = nc
    return _program_cache[key]


def kernel(history_item_ids, history_item_masks, embedding_table, code_book,
           W_enc, b_enc):
    ids = np.asarray(history_item_ids)
    masks = np.asarray(history_item_masks)
    E = np.asarray(embedding_table, dtype=np.float32)
    C = np.asarray(code_book, dtype=np.float32)
    W = np.asarray(W_enc, dtype=np.float32)
    b = np.asarray(b_enc, dtype=np.float32)
    assert ids.shape == (B, L) and E.shape == (V, D) and C.shape == (K, D)

    mask = (masks >= 1)
    cnt = mask.sum(axis=1).astype(np.float64)                   # [B]
    n_act = [int(mask[c * BL:(c + 1) * BL].sum()) for c in range(N_CORES)]
    T = max(1, -(-max(n_act) // 128))

    norms = (C ** 2).sum(axis=1, dtype=np.float32)              # fp32 like ref
    n0 = int(np.argmin(norms))

    sc = np.float32(SCALE)
    Emb_scaled = E * sc
    Cs = C * sc
    Ch = Cs.astype(np.float16)
    Cl = (Cs - Ch.astype(np.float32)).astype(np.float16)

    normsb_row = (norms.astype(np.float64) * float(sc) ** 2 / 2.0).astype(np.float32)
    normsb = np.broadcast_to(normsb_row, (128, K)).copy()

    colsum = C.sum(axis=0, dtype=np.float64)
    bias_eff = (b.astype(np.float64) + colsum @ W[0:D].astype(np.float64)
                ).astype(np.float32)
    bias_bcast = np.broadcast_to(bias_eff, (BL, D)).copy()

    wT = np.zeros((128, XC * D), np.float32)
    for c in range(XC):
        wT[:, c * D:(c + 1) * D] = W[c * 128:(c + 1) * 128]

    chT = np.zeros((128, KC * K), np.float16)
    clT = np.zeros((128, KC * K), np.float16)
    for kc in range(KC):
        chT[:, kc * K:(kc + 1) * K] = Ch[:, kc * 128:(kc + 1) * 128].T
        clT[:, kc * K:(kc + 1) * K] = Cl[:, kc * 128:(kc + 1) * 128].T
    ch2 = np.zeros((128, SC * D), np.float16)
    cl2 = np.zeros((128, SC * D), np.float16)
    for kc in range(SC):
        ch2[:, kc * D:(kc + 1) * D] = Ch[kc * 128:(kc + 1) * 128]
        cl2[:, kc * D:(kc + 1) * D] = Cl[kc * 128:(kc + 1) * 128]

    R = 128 * T
    in_maps = []
    for core in range(N_CORES):
        bsl = slice(core * BL, (core + 1) * BL)
        ids_c = ids[bsl]
        mask_c = mask[bsl]
        cnt_c = cnt[bsl]

        act_b, act_l = np.nonzero(mask_c)
        na = act_b.shape[0]
        assert na <= R
        ids_packed = np.zeros(R, np.int32)
        ids_packed[:na] = ids_c[act_b, act_l]
        memb = np.full((R,), -1, np.int64)
        memb[:na] = act_b

        ids_tile = np.zeros((128, T), np.int32)
        S = np.zeros((128, T * BL), np.float16)
        for t in range(T):
            rows = np.arange(t * 128, (t + 1) * 128)
            ids_tile[:, t] = ids_packed[rows]
            mb = memb[rows]
            valid = mb >= 0
            S[np.nonzero(valid)[0], t * BL + mb[valid]] = 1.0

        g01 = np.zeros((128, 2 * D), np.float32)
        for t in range(min(2, T)):
            g01[:, t * D:(t + 1) * D] = Emb_scaled[ids_tile[:, t]]
        in_maps.append({
            "emb": Emb_scaled,
            "g01": g01,
            "ids": ids_tile,
            "s16": S,
            "normsb": normsb,
            "chT": chT, "clT": clT, "ch2": ch2, "cl2": cl2,
            "rvq": (1.0 / float(sc) / cnt_c).astype(np.float32).reshape(BL, 1),
            "rhist": (1.0 / float(sc) / (cnt_c + 1e-9)).astype(np.float32
                                                              ).reshape(BL, 1),
            "corr": (L - cnt_c).astype(np.float32).reshape(BL, 1),
            "wT": wT,
            "bias": bias_bcast,
        })

    nc = _get_program(T, n0)
    res = run_bass_kernel_spmd(nc, in_maps, core_ids=list(range(N_CORES)))
    return np.concatenate([res.results[c]["out"] for c in range(N_CORES)],
                          axis=0)

